# revision 3
# baseline (speedup 1.0000x reference)
"""MoE FFN (grouped top-1 routing, SwiGLU experts) on 8 Trainium2 NeuronCores.

Strategy (expert-parallel, per sharding hint):
  - Host computes the (tiny) routers: sigmoid(x @ macro_w) -> top-1 group of 4;
    within the selected group both 2 experts are active (TOP_K==EXPERTS_PER_GROUP)
    with sigmoid-normalized weights. Router cost is ~25 MFLOP -> negligible.
  - Tokens are dispatched by routed group ("all-to-all" staged host-side into
    per-core input maps). Core c owns expert c (group c//2); it receives the
    tokens of its group, padded to capacity C, plus its expert's weights.
  - Per-expert weight w[t,e] is folded into the up-projection input on the host
    (x*w), so the device output is already weighted; host just adds the two
    expert partials of each group and scatters back to token order.
  - Device kernel: Y^T = down^T @ (silu(gate^T X^T) * (up^T Xw^T)), all with
    features on SBUF partitions and tokens on the free dim, fp32 storage with
    float32r (FP22) matmuls on the PE.
"""

import math

import numpy as np

import concourse.bass as bass  # noqa: F401  (bass types via bacc)
import concourse.mybir as mybir
import concourse.tile as tile
from concourse import bacc
from concourse.bass_utils import run_bass_kernel_spmd

P = 128
D_MODEL = 1024
FFN_DIM = 2048
NUM_EXPERTS = 8
NUM_GROUPS = 4
EPS = 1e-9

F32 = mybir.dt.float32
F32R = mybir.dt.float32r

N_CORES = 8
C_CAP = 1024  # max token capacity per core per round (SBUF-bounded)

_BUILD_CACHE: dict[int, object] = {}
LAST_RESULTS = None  # stashed BassKernelResults for test harnesses


def _build(C: int, nch: int):
    """Bass/Tile program for one expert: [D,C]x2 tokens + expert weights -> [D,C]."""
    chunk = C // nch
    assert chunk * nch == C and chunk <= 512
    DO = D_MODEL // P  # 8 k-tiles over D
    FO = FFN_DIM // P  # 16 f-tiles over F

    nc = bacc.Bacc(
        "TRN2",
        target_bir_lowering=False,
        debug=False,
        enable_asserts=False,
        num_devices=N_CORES,
    )
    xt = nc.dram_tensor("xt", [D_MODEL, C], F32R, kind="ExternalInput").ap()
    xwt = nc.dram_tensor("xwt", [D_MODEL, C], F32R, kind="ExternalInput").ap()
    gw = nc.dram_tensor("gw", [D_MODEL, FFN_DIM], F32R, kind="ExternalInput").ap()
    uw = nc.dram_tensor("uw", [D_MODEL, FFN_DIM], F32R, kind="ExternalInput").ap()
    dw = nc.dram_tensor("dw", [FFN_DIM, D_MODEL], F32R, kind="ExternalInput").ap()
    yt = nc.dram_tensor("yt", [D_MODEL, C], F32, kind="ExternalOutput").ap()

    gwr = gw.rearrange("(do p) f -> p do f", p=P)
    uwr = uw.rearrange("(do p) f -> p do f", p=P)
    dwr = dw.rearrange("(fo p) d -> p fo d", p=P)
    xtr = xt.rearrange("(do p) c -> p do c", p=P)
    xwtr = xwt.rearrange("(do p) c -> p do c", p=P)

    with tile.TileContext(nc) as tc:
        with (
            tc.tile_pool(name="xp", bufs=1) as xp,
            tc.tile_pool(name="wp", bufs=3) as wp,
            tc.tile_pool(name="dp", bufs=2) as dp,
            tc.tile_pool(name="hp", bufs=1) as hp,
            tc.tile_pool(name="sp", bufs=4) as sp,
            tc.tile_pool(name="yp", bufs=4) as yp,
            tc.tile_pool(name="pg", bufs=2, space="PSUM") as pgp,
            tc.tile_pool(name="pu", bufs=2, space="PSUM") as pup,
            tc.tile_pool(name="pd", bufs=2, space="PSUM") as pdp,
        ):
            xts = xp.tile([P, DO, C], F32R, tag="xt")
            xws = xp.tile([P, DO, C], F32R, tag="xw")
            for do in range(DO):
                nc.sync.dma_start(xts[:, do], xtr[:, do])
                nc.sync.dma_start(xws[:, do], xwtr[:, do])
            hs = hp.tile([P, FO, C], F32R, tag="h")

            for fo in range(FO):
                gt = wp.tile([P, DO, P], F32R, tag="gt")
                nc.sync.dma_start(gt[:], gwr[:, :, fo * P : (fo + 1) * P])
                ut = wp.tile([P, DO, P], F32R, tag="ut")
                nc.sync.dma_start(ut[:], uwr[:, :, fo * P : (fo + 1) * P])
                for cc in range(nch):
                    cs = slice(cc * chunk, (cc + 1) * chunk)
                    psg = pgp.tile([P, chunk], F32, tag="psg")
                    psu = pup.tile([P, chunk], F32, tag="psu")
                    for do in range(DO):
                        nc.tensor.matmul(
                            psg[:],
                            gt[:, do],
                            xts[:, do, cs],
                            start=(do == 0),
                            stop=(do == DO - 1),
                        )
                    for do in range(DO):
                        nc.tensor.matmul(
                            psu[:],
                            ut[:, do],
                            xws[:, do, cs],
                            start=(do == 0),
                            stop=(do == DO - 1),
                        )
                    sg = sp.tile([P, chunk], F32, tag="sg")
                    nc.scalar.activation(
                        sg[:], psg[:], mybir.ActivationFunctionType.Silu
                    )
                    nc.vector.tensor_mul(out=hs[:, fo, cs], in0=sg[:], in1=psu[:])

            for do in range(DO):
                dt_ = dp.tile([P, FO, P], F32R, tag="dt")
                nc.sync.dma_start(dt_[:], dwr[:, :, do * P : (do + 1) * P])
                for cc in range(nch):
                    cs = slice(cc * chunk, (cc + 1) * chunk)
                    psy = pdp.tile([P, chunk], F32, tag="psy")
                    for fo in range(FO):
                        nc.tensor.matmul(
                            psy[:],
                            dt_[:, fo],
                            hs[:, fo, cs],
                            start=(fo == 0),
                            stop=(fo == FO - 1),
                        )
                    yo = yp.tile([P, chunk], F32, tag="yo")
                    nc.any.tensor_copy(out=yo[:], in_=psy[:])
                    nc.sync.dma_start(yt[do * P : (do + 1) * P, cs], yo[:])
    nc.finalize()
    return nc


def _get_program(C: int, nch: int):
    key = (C, nch)
    if key not in _BUILD_CACHE:
        _BUILD_CACHE[key] = _build(C, nch)
    return _BUILD_CACHE[key]


def _sigmoid(z):
    return 1.0 / (1.0 + np.exp(-z))


def _route(xf32, macro_w, micro_w):
    """Host routers in float64. Returns group index per token and per-token
    weights for the 2 experts of the selected group (float32)."""
    xf = xf32.astype(np.float64)
    ms = _sigmoid(xf @ macro_w.astype(np.float64))  # [T, G]
    g_sel = np.argmax(ms, axis=1)
    T = xf.shape[0]
    mval = ms[np.arange(T), g_sel]
    mv = mval / (mval + EPS)

    w2 = np.zeros((T, 2), np.float64)
    for g in range(NUM_GROUPS):
        idx = np.nonzero(g_sel == g)[0]
        if idx.size == 0:
            continue
        s = _sigmoid(xf[idx] @ micro_w[g].astype(np.float64))  # [n, 2]
        denom = np.maximum(s[:, 0], s[:, 1]) + np.minimum(s[:, 0], s[:, 1]) + EPS
        w2[idx, 0] = mv[idx] * s[:, 0] / denom
        w2[idx, 1] = mv[idx] * s[:, 1] / denom
    return g_sel, w2.astype(np.float32)


def _pick_capacity(n: int):
    n = max(n, 64)
    nch = (n + 511) // 512
    chunk = -(-n // nch)
    chunk = -(-chunk // 16) * 16
    return chunk * nch, nch


def kernel(x, macro_w, micro_w, gate_w, up_w, down_w):
    global LAST_RESULTS
    x = np.asarray(x)
    B, S, D = x.shape
    T = B * S
    xf = np.ascontiguousarray(x.reshape(T, D).astype(np.float32, copy=False))

    g_sel, w2 = _route(xf, np.asarray(macro_w), np.asarray(micro_w))
    idx_by_g = [np.nonzero(g_sel == g)[0] for g in range(NUM_GROUPS)]
    max_n = max(ix.size for ix in idx_by_g)

    n_rounds = max(1, math.ceil(max_n / C_CAP))
    if n_rounds > 1:
        C, nch = C_CAP, 2
    else:
        C, nch = _pick_capacity(max_n)
    nc = _get_program(C, nch)

    gate_w = np.ascontiguousarray(np.asarray(gate_w, np.float32))
    up_w = np.ascontiguousarray(np.asarray(up_w, np.float32))
    down_w = np.ascontiguousarray(np.asarray(down_w, np.float32))

    y = np.zeros((T, D), np.float32)
    for r in range(n_rounds):
        in_maps = []
        round_idx = []
        for c in range(N_CORES):
            g = c // 2
            j = c % 2  # local expert within group
            ix = idx_by_g[g][r * C_CAP : r * C_CAP + C]
            round_idx.append(ix)
            xt = np.zeros((D, C), np.float32)
            xwt = np.zeros((D, C), np.float32)
            if ix.size:
                xg = xf[ix]
                xt[:, : ix.size] = xg.T
                xwt[:, : ix.size] = (xg * w2[ix, j : j + 1]).T
            in_maps.append(
                {
                    "xt": xt,
                    "xwt": xwt,
                    "gw": gate_w[c],
                    "uw": up_w[c],
                    "dw": down_w[c],
                }
            )
        res = run_bass_kernel_spmd(nc, in_maps, core_ids=list(range(N_CORES)))
        LAST_RESULTS = res
        for g in range(NUM_GROUPS):
            ix = round_idx[2 * g]
            if ix.size:
                ysum = res.results[2 * g]["yt"] + res.results[2 * g + 1]["yt"]
                y[ix] = ysum[:, : ix.size].T
    return y.reshape(B, S, D)
